# revision 23
# baseline (speedup 1.0000x reference)
"""Trainium2 Bass kernel for nn_Despawn2D: 8-level 1-D circular-conv DWT
(forward + inverse) over 2048 rows, data-parallel across 8 NeuronCores.

Self-contained: hardcodes shapes (input [2048, 3000] f32, filters [8, 8]).

Per core (256 rows):
  - DMA rows in, edge-pad to 4096 in row-major SBUF, PE-transpose to
    "signal-on-partitions" layout [4096, 256] stored even/odd-block split.
  - Forward: banded conv + downsample as 128x128 block matmuls (fp32r,
    free dim 512 via the even/odd split), accumulating in PSUM.
  - Inverse: upsample + conv as block matmuls, d+a fused in PSUM.
  - PE-transpose coeffs/recon back to row-major, DMA out.
"""
import numpy as np

TAPS = 8
LEVELS = 8
L = 4096
P0 = 548
M_IN = 3000
R = 256            # rows per core
NCORES = 8
BIG_LVLS = 5       # levels 0..4 use block weights; 5..7 circular


# ---------------------------------------------------------------------------
# host-side weight construction
# ---------------------------------------------------------------------------

def _make_wavelet(h):
    g = np.asarray(h, dtype=np.float32)[::-1].copy()
    g[1::2] *= -1
    return g


def _fwd_block_weights(f):
    Wm = np.zeros((128, 128), np.float32)
    Wn = np.zeros((128, 128), np.float32)
    Wp = np.zeros((128, 128), np.float32)          # rows 121..127 active
    for t in range(128):
        for p in range(128):
            j = 2 * t - p
            if 0 <= j < TAPS:
                Wm[p, t] = f[j]
            j = 2 * t - p - 128
            if 0 <= j < TAPS:
                Wn[p, t] = f[j]
            j = 2 * t - p + 128
            if 0 <= j < TAPS:
                Wp[p, t] = f[j]
    return Wm, Wn, Wp


def _fwd_circ_weight(f, N):
    W = np.zeros((N, N // 2), np.float32)
    for t in range(N // 2):
        for j in range(TAPS):
            W[(2 * t - j) % N, t] += f[j]
    return W


def _inv_block_weights(f):
    We = np.zeros((128, 128), np.float32)
    Wo = np.zeros((128, 128), np.float32)
    Ww = np.zeros((128, 128), np.float32)          # rows 0..3 active
    for t in range(128):
        for p in range(128):
            j = 2 * p - t
            if 0 <= j < TAPS:
                We[p, t] = f[j]
            j = 2 * p - t - 128
            if 0 <= j < TAPS:
                Wo[p, t] = f[j]
            j = 2 * p - t + 128
            if 0 <= j < TAPS:
                Ww[p, t] = f[j]
    return We, Wo, Ww


def _inv_circ_weight(f, N):
    K = N // 2
    W = np.zeros((K, N), np.float32)
    for m in range(K):
        for n in range(N):
            if (2 * m - n) % N < TAPS:
                W[m, n] += f[(2 * m - n) % N]
    return W


def build_weights(scaling, scaling_rec):
    """Packed [128, NSLOTS*128] array (identity in last slot) + slot index."""
    filts = []
    for lvl in range(LEVELS):
        filts.append((_make_wavelet(scaling_rec[lvl]),
                      np.asarray(scaling[lvl], np.float32)))
    slots, index, seen = [], {}, {}

    def add(key, W):
        W = np.ascontiguousarray(W.astype(np.float32))
        kb = (W.shape, W.tobytes())
        if kb in seen:
            index[key] = seen[kb]
            return
        seen[kb] = len(slots)
        index[key] = len(slots)
        slots.append(W)

    add(("id",), np.eye(128, dtype=np.float32))
    for lvl in range(BIG_LVLS):
        for fi in range(2):
            Wm, Wn, Wp = _fwd_block_weights(filts[lvl][fi])
            add(("fm", lvl, fi), Wm)
            add(("fn", lvl, fi), Wn)
            add(("fp", lvl, fi), Wp)
    W5d = _fwd_circ_weight(filts[5][0], 128)
    W5a = _fwd_circ_weight(filts[5][1], 128)
    W6d = _fwd_circ_weight(filts[6][0], 64)
    W6a = _fwd_circ_weight(filts[6][1], 64)
    W7d = _fwd_circ_weight(filts[7][0], 32)
    W7a = _fwd_circ_weight(filts[7][1], 32)
    IC5d = _inv_circ_weight(filts[5][0], 128)
    IC5a = _inv_circ_weight(filts[5][1], 128)
    IC6d = _inv_circ_weight(filts[6][0], 64)
    IC6a = _inv_circ_weight(filts[6][1], 64)
    IC7d = _inv_circ_weight(filts[7][0], 32)
    IC7a = _inv_circ_weight(filts[7][1], 32)
    Wcomp = np.hstack([W5d, W5a @ W6d, W5a @ W6a @ W7d, W5a @ W6a @ W7a])
    M5 = (W5d @ IC5d + W5a @ W6d @ IC6d @ IC5a
          + W5a @ W6a @ W7d @ IC7d @ IC6a @ IC5a
          + W5a @ W6a @ W7a @ IC7a @ IC6a @ IC5a)
    D4 = _fwd_circ_weight(filts[4][0], 256)
    A5w = _fwd_circ_weight(filts[4][1], 256)
    IV4d = _inv_circ_weight(filts[4][0], 256)
    IV4a = _inv_circ_weight(filts[4][1], 256)
    C4 = A5w @ Wcomp
    M4 = D4 @ IV4d + A5w @ M5 @ IV4a
    for b in range(2):
        add(("d4", b), D4[128 * b:128 * b + 128])
        add(("c4", b), C4[128 * b:128 * b + 128])
        for h in range(2):
            add(("m4", b, h), M4[128 * b:128 * b + 128, 128 * h:128 * h + 128])
    nfwd = len(slots)
    for lvl in range(BIG_LVLS):
        for fi in range(2):
            We, Wo, Ww = _inv_block_weights(filts[lvl][fi])
            add(("ie", lvl, fi), We)
            add(("io", lvl, fi), Wo)
            add(("iw", lvl, fi), Ww)
    nslots = len(slots)
    packed = np.zeros((128, nslots * 128), np.float32)
    for i, W in enumerate(slots):
        K, M = W.shape
        packed[:K, i * 128:i * 128 + M] = W
    return packed, index, nslots, nfwd


# ---------------------------------------------------------------------------
# bass kernel
# ---------------------------------------------------------------------------

_CACHED = {}


def _build_nc(nslots, nfwd, windex):
    from contextlib import ExitStack
    import concourse.tile as tile
    from concourse import bacc, mybir

    F32 = mybir.dt.float32
    F32R = mybir.dt.float32r

    nc = bacc.Bacc("TRN2", target_bir_lowering=False, debug=False)
    x_d = nc.dram_tensor("x", (R, M_IN), F32R, kind="ExternalInput").ap()
    w_d = nc.dram_tensor("wts", (128, nslots * 128), F32R, kind="ExternalInput").ap()
    rec_d = nc.dram_tensor("recon", (R, M_IN), F32, kind="ExternalOutput").ap()
    co_d = nc.dram_tensor("coeffs", (R, L), F32, kind="ExternalOutput").ap()

    with tile.TileContext(nc) as tc, ExitStack() as ctx:
        wt_pool = ctx.enter_context(tc.tile_pool(name="wt", bufs=1))
        rm_pool = ctx.enter_context(tc.tile_pool(name="rm", bufs=2))
        xt_pool = ctx.enter_context(tc.tile_pool(name="xt", bufs=1))
        det_pool = ctx.enter_context(tc.tile_pool(name="det", bufs=1))
        app_pool = ctx.enter_context(tc.tile_pool(name="app", bufs=1))
        ia_pool = ctx.enter_context(tc.tile_pool(name="ia", bufs=1))
        win_pool = ctx.enter_context(tc.tile_pool(name="win", bufs=8))
        stg_pool = ctx.enter_context(tc.tile_pool(name="stg", bufs=4))
        cps_pool = ctx.enter_context(tc.tile_pool(name="cps", bufs=4, space="PSUM"))
        tps_pool = ctx.enter_context(tc.tile_pool(name="tps", bufs=4, space="PSUM"))

        wtile = wt_pool.tile([128, nslots * 128], F32R, tag="wt")
        # identity (slot 0) first: the in-transposes need it immediately
        nc.sync.dma_start(wtile[:, 0:128], w_d[:, 0:128])

        def wslot(key, rows=128, cols=128):
            s = windex[key]
            return wtile[0:rows, s * 128: s * 128 + cols]

        identR = wslot(("id",))

        dr_tgl = [0]

        def drain(dst, src):
            # alternate ACT / DVE to balance PSUM-drain load
            if dr_tgl[0] % 2 == 0:
                nc.scalar.copy(dst, src)
            else:
                nc.vector.tensor_copy(dst, src)
            dr_tgl[0] += 1

        def tposed(tp_slice, src, first, last, kk=128):
            nc.tensor.matmul(tp_slice.bitcast(F32R), src, identR[0:kk, 0:kk],
                             is_transpose=True, start=first, stop=last)

        # ------------------------------------------------------ phase A: load
        # xT: level-0 signal, even blocks at slots 0..15, odd at 16..31
        xT = xt_pool.tile([128, 32 * R], F32R, tag="xt")
        rms = []
        for g in range(2):
            rm = rm_pool.tile([128, L], F32R, tag="rm")
            half = M_IN // 2
            nc.sync.dma_start(rm[:, P0:P0 + half],
                              x_d[g * 128:(g + 1) * 128, 0:half])
            nc.scalar.dma_start(rm[:, P0 + half:P0 + M_IN],
                                x_d[g * 128:(g + 1) * 128, half:M_IN])
            nc.vector.tensor_copy(rm[:, 512:P0],
                                  rm[:, P0:P0 + 1].broadcast_to([128, P0 - 512]))
            nc.vector.tensor_copy(rm[:, P0 + M_IN:3584],
                                  rm[:, P0 + M_IN - 1:P0 + M_IN]
                                  .broadcast_to([128, 3584 - P0 - M_IN]))
            nc.vector.tensor_copy(rm[:, 0:512],
                                  rm[:, P0:P0 + 1].broadcast_to([128, 512]))
            nc.vector.tensor_copy(rm[:, 3584:L],
                                  rm[:, P0 + M_IN - 1:P0 + M_IN]
                                  .broadcast_to([128, L - 3584]))
            rms.append(rm)
        nc.scalar.dma_start(wtile[:, 128:nfwd * 128], w_d[:, 128:nfwd * 128])
        nc.scalar.dma_start(wtile[:, nfwd * 128:], w_d[:, nfwd * 128:])
        for g in range(2):
            for c in (2, 3, 1, 4, 5, 6, 0, 7):
                rm = rms[g]
                tp = tps_pool.tile([128, 512], F32, tag="tps")
                order = (4 * c, 4 * c + 2, 4 * c + 1, 4 * c + 3)  # E,E,O,O
                for i, b in enumerate(order):
                    tposed(tp[:, i * 128:(i + 1) * 128],
                           rm[:, b * 128:(b + 1) * 128], i == 0, i == 3)
                for h in range(2):
                    out = xT[:, h * 16 * R: (h + 1) * 16 * R] \
                        .rearrange("p (s x) -> p s x", s=16)[:, 2 * c:2 * c + 2,
                                                            g * 128:(g + 1) * 128]
                    inp = tp[:, h * 256:(h + 1) * 256] \
                        .rearrange("p (s x) -> p s x", s=2)
                    drain(out, inp)

        # coeff chunk emitter: chunk c covers coeff position blocks 4c..4c+3
        cblocks = []
        for lvl in range(BIG_LVLS):
            for b in range((2048 >> lvl) // 128):
                cblocks.append(("det", lvl, b))
        cblocks.append(("comp", None, None))
        assert len(cblocks) == 32

        def emit_coeff_chunk(c, g, dets, app8):
            tp = tps_pool.tile([128, 512], F32, tag="tps")
            nt = 0
            for i in range(4):
                kind, lvl, b = cblocks[4 * c + i]
                if kind == "det":
                    tposed(tp[:, i * 128:(i + 1) * 128],
                           dets[lvl][0][:, b * R + g * 128:
                                        b * R + (g + 1) * 128],
                           nt == 0, i == 3)
                    nt += 1
                else:
                    tposed(tp[:, i * 128:(i + 1) * 128],
                           app8[:, g * 128:(g + 1) * 128],
                           nt == 0, i == 3)
                    nt += 1
            stg = stg_pool.tile([128, 512], F32, tag="stg")
            drain(stg[:], tp[:])
            nc.sync.dma_start(co_d[g * 128:(g + 1) * 128,
                                   512 * c:512 * (c + 1)], stg[:, :])

        # --------------------------------------------------- phase B: forward
        dets = []
        cur = xT            # eo layout, fp32r
        for lvl in range(4):
            N = L >> lvl
            no = N // 2
            det = det_pool.tile([min(128, no), max(no // 128, 1) * R], F32R,
                                tag=f"det{lvl}")
            app = app_pool.tile([min(128, no), max(no // 128, 1) * R], F32R,
                                tag=f"app{lvl+1}")
            if True:
                nbO = N // 256          # blocks per E/O region
                xE = cur[:, 0:nbO * R]
                xO = cur[:, nbO * R:2 * nbO * R]
                P = max(N // 512, 1)
                w = 512 if N >= 512 else 256
                if lvl == 0:
                    q_order = [2, 3, 4, 5, 6, 1, 7, 0]
                else:
                    q_order = list(range(1, P)) + [0]
                for fi, dst in ((0, det), (1, app)):
                    for q in q_order:
                        ps = cps_pool.tile([128, 512], F32, tag="cps")
                        ps_u = ps[:, 0:w]
                        nc.tensor.matmul(ps_u, wslot(("fm", lvl, fi)),
                                         xE[:, 2 * q * R: 2 * q * R + w],
                                         start=True, stop=False)
                        nc.tensor.matmul(ps_u, wslot(("fn", lvl, fi)),
                                         xO[:, 2 * q * R: 2 * q * R + w],
                                         start=False, stop=False)
                        if q == 0:
                            nc.tensor.matmul(ps[:, 0:256], wslot(("fp", lvl, fi)),
                                             xO[:, (nbO - 1) * R: nbO * R],
                                             start=False, stop=(w == 256))
                            if w == 512:
                                nc.tensor.matmul(ps[:, 256:512],
                                                 wslot(("fp", lvl, fi)),
                                                 xO[:, 0:R],
                                                 start=False, stop=True)
                        else:
                            nc.tensor.matmul(ps_u, wslot(("fp", lvl, fi)),
                                             xO[:, (2 * q - 1) * R:
                                                (2 * q + 1) * R],
                                             start=False, stop=True)
                        if fi == 0:
                            drain(det[:, 2 * q * R: 2 * q * R + w], ps_u)
                        elif (lvl + 1) < BIG_LVLS:
                            # app is eo: halves go to E/O regions, slot q
                            nbO2 = no // 256
                            out = app[:].rearrange("p (h s x) -> p h s x",
                                                   h=2, s=nbO2)
                            drain(out[:, :, q, :],
                                  ps_u.rearrange("p (h x) -> p h x", h=2))
                        else:
                            drain(app[:, 0:w], ps_u)
            dets.append((det, no))
            cur = app

        app4 = cur          # eo layout [128, 2*R]: E = [0:R], O = [R:2R]
        # composed levels 4..7: det4, [det5|det6|det7|app8], and the
        # reconstructed level-4 approx, each straight from app4
        ia4 = ia_pool.tile([128, 2 * R], F32R, tag="ia4")
        ps = cps_pool.tile([128, 512], F32, tag="cps")
        for h in range(2):
            for b in range(2):
                nc.tensor.matmul(ps[:, h * R:(h + 1) * R], wslot(("m4", b, h)),
                                 app4[:, b * R:(b + 1) * R],
                                 start=(h == 0 and b == 0),
                                 stop=(h == 1 and b == 1))
        drain(ia4[:], ps[:])
        det4 = det_pool.tile([128, R], F32R, tag="det4")
        ps = cps_pool.tile([128, 512], F32, tag="cps")
        nc.tensor.matmul(ps[:, 0:R], wslot(("d4", 0)), app4[:, 0:R],
                         start=True, stop=False)
        nc.tensor.matmul(ps[:, 0:R], wslot(("d4", 1)), app4[:, R:2 * R],
                         start=False, stop=True)
        drain(det4[:], ps[:, 0:R])
        dets.append((det4, 128))
        comp = det_pool.tile([128, R], F32R, tag="comp")
        ps = cps_pool.tile([128, 512], F32, tag="cps")
        nc.tensor.matmul(ps[:, 0:R], wslot(("c4", 0)), app4[:, 0:R],
                         start=True, stop=False)
        nc.tensor.matmul(ps[:, 0:R], wslot(("c4", 1)), app4[:, R:2 * R],
                         start=False, stop=True)
        drain(comp[:], ps[:, 0:R])

        for g in range(2):
            for cch in range(8):
                emit_coeff_chunk(cch, g, dets, comp)

        # --------------------------------------------------- phase C: inverse
        a = ia4
        for lvl in range(3, -1, -1):
            N = L >> lvl
            K = N // 2
            d = dets[lvl][0]
            nb = N // 128
            nbK = max(K // 128, 1)
            if lvl > 0:
                out = ia_pool.tile([128, nb * R], F32R, tag=f"ia{lvl}")
                outv = out[:].rearrange("p (s x) -> p s x", s=nb)
            if N == 256:
                ps = cps_pool.tile([128, 512], F32, tag="cps")
                nc.tensor.matmul(ps[:, 0:256], wslot(("ie", lvl, 0)),
                                 d[:, 0:R], start=True, stop=False)
                nc.tensor.matmul(ps[:, 0:256], wslot(("ie", lvl, 1)),
                                 a[:, 0:R], start=False, stop=False)
                nc.tensor.matmul(ps[:, 256:512], wslot(("io", lvl, 0)),
                                 d[:, 0:R], start=False, stop=False)
                nc.tensor.matmul(ps[:, 256:512], wslot(("io", lvl, 1)),
                                 a[:, 0:R], start=False, stop=False)
                nc.tensor.matmul(ps[:, 256:512], wslot(("iw", lvl, 0)),
                                 d[:, 0:R], start=False, stop=False)
                nc.tensor.matmul(ps[:, 256:512], wslot(("iw", lvl, 1)),
                                 a[:, 0:R], start=False, stop=True)
                drain(out[:, 0:512], ps[:])
                a = out
                continue
            qr = range(1, 7) if lvl == 0 else range(nb // 4)
            for q in qr:
                # even pair: out blocks (4q, 4q+2) <- d/a blocks 2q, 2q+1
                pse = cps_pool.tile([128, 512], F32, tag="cps")
                nc.tensor.matmul(pse[:], wslot(("ie", lvl, 0)),
                                 d[:, 2 * q * R:(2 * q + 2) * R],
                                 start=True, stop=False)
                nc.tensor.matmul(pse[:], wslot(("ie", lvl, 1)),
                                 a[:, 2 * q * R:(2 * q + 2) * R],
                                 start=False, stop=True)
                # odd pair: out blocks (4q+1, 4q+3)
                pso = cps_pool.tile([128, 512], F32, tag="cps")
                nc.tensor.matmul(pso[:], wslot(("io", lvl, 0)),
                                 d[:, 2 * q * R:(2 * q + 2) * R],
                                 start=True, stop=False)
                nc.tensor.matmul(pso[:], wslot(("io", lvl, 1)),
                                 a[:, 2 * q * R:(2 * q + 2) * R],
                                 start=False, stop=False)
                if 2 * q + 2 < nbK:
                    nc.tensor.matmul(pso[:], wslot(("iw", lvl, 0)),
                                     d[:, (2 * q + 1) * R:(2 * q + 3) * R],
                                     start=False, stop=False)
                    nc.tensor.matmul(pso[:], wslot(("iw", lvl, 1)),
                                     a[:, (2 * q + 1) * R:(2 * q + 3) * R],
                                     start=False, stop=True)
                else:
                    nc.tensor.matmul(pso[:, 0:256], wslot(("iw", lvl, 0)),
                                     d[:, (2 * q + 1) * R:(2 * q + 2) * R],
                                     start=False, stop=False)
                    nc.tensor.matmul(pso[:, 0:256], wslot(("iw", lvl, 1)),
                                     a[:, (2 * q + 1) * R:(2 * q + 2) * R],
                                     start=False, stop=False)
                    nc.tensor.matmul(pso[:, 256:512], wslot(("iw", lvl, 0)),
                                     d[:, 0:R], start=False, stop=False)
                    nc.tensor.matmul(pso[:, 256:512], wslot(("iw", lvl, 1)),
                                     a[:, 0:R], start=False, stop=True)
                if lvl > 0:
                    drain(outv[:, 4 * q:4 * q + 3:2, :],
                          pse[:].rearrange("p (s x) -> p s x", s=2))
                    drain(outv[:, 4 * q + 1:4 * q + 4:2, :],
                          pso[:].rearrange("p (s x) -> p s x", s=2))
                else:
                    we = win_pool.tile([128, 512], F32R, tag="win")
                    wo = win_pool.tile([128, 512], F32R, tag="win")
                    drain(we[:], pse[:])
                    drain(wo[:], pso[:])
                    for g in range(2):
                        tp = tps_pool.tile([128, 512], F32, tag="tps")
                        srcs = [we[:, g * 128:g * 128 + 128],
                                wo[:, g * 128:g * 128 + 128],
                                we[:, 256 + g * 128:256 + g * 128 + 128],
                                wo[:, 256 + g * 128:256 + g * 128 + 128]]
                        for i, s in enumerate(srcs):
                            tposed(tp[:, i * 128:(i + 1) * 128], s,
                                   i == 0, i == 3)
                        stg = stg_pool.tile([128, 512], F32, tag="stg")
                        drain(stg[:], tp[:])
                        lo = 512 * q
                        s0 = max(P0 - lo, 0)
                        s1 = min(P0 + M_IN - lo, 512)
                        nc.sync.dma_start(
                            rec_d[g * 128:(g + 1) * 128,
                                  lo + s0 - P0: lo + s1 - P0],
                            stg[:, s0:s1])
            if lvl > 0:
                a = out

    nc.compile()
    return nc


def kernel(input, scaling, scaling_rec):
    from concourse import bass_utils

    x = np.ascontiguousarray(np.asarray(input, dtype=np.float32))
    packed, windex, nslots, nfwd = build_weights(
        np.asarray(scaling, np.float32), np.asarray(scaling_rec, np.float32))
    key = (nslots, nfwd, tuple(sorted((k, v) for k, v in windex.items())))
    if key not in _CACHED:
        _CACHED[key] = _build_nc(nslots, nfwd, windex)
    nc = _CACHED[key]

    in_maps = []
    for c in range(NCORES):
        in_maps.append({"x": np.ascontiguousarray(x[c * R:(c + 1) * R]),
                        "wts": packed})
    res = bass_utils.run_bass_kernel_spmd(nc, in_maps, core_ids=list(range(NCORES)))
    recon = np.concatenate([r["recon"] for r in res.results], axis=0)
    coeffs = np.concatenate([r["coeffs"] for r in res.results], axis=0)
    return recon, coeffs


if __name__ == "__main__":
    rng = np.random.default_rng(0)
    x = rng.standard_normal((NCORES * R, M_IN), dtype=np.float32)
    DEC_HI = np.array([-0.23037781330885523, 0.7148465705525415, -0.6308807679295904,
                       -0.02798376941698385, 0.18703481171888114, 0.030841381835986965,
                       -0.032883011666982945, -0.010597401784997278], dtype=np.float32)
    sc = np.tile(DEC_HI, (LEVELS, 1))
    scr = np.tile(DEC_HI[::-1], (LEVELS, 1))
    r, c = kernel(x, sc, scr)
    print("recon", r.shape, "coeffs", c.shape)


# revision 24
# speedup vs baseline: 1.0091x; 1.0091x over previous
"""Trainium2 Bass kernel for nn_Despawn2D: 8-level 1-D circular-conv DWT
(forward + inverse) over 2048 rows, data-parallel across 8 NeuronCores.

Self-contained: hardcodes shapes (input [2048, 3000] f32, filters [8, 8]).

Per core (256 rows):
  - DMA rows in, edge-pad to 4096 in row-major SBUF, PE-transpose to
    "signal-on-partitions" layout [4096, 256] stored even/odd-block split.
  - Forward: banded conv + downsample as 128x128 block matmuls (fp32r,
    free dim 512 via the even/odd split), accumulating in PSUM.
  - Inverse: upsample + conv as block matmuls, d+a fused in PSUM.
  - PE-transpose coeffs/recon back to row-major, DMA out.
"""
import numpy as np

TAPS = 8
LEVELS = 8
L = 4096
P0 = 548
M_IN = 3000
R = 256            # rows per core
NCORES = 8
BIG_LVLS = 5       # levels 0..4 use block weights; 5..7 circular


# ---------------------------------------------------------------------------
# host-side weight construction
# ---------------------------------------------------------------------------

def _make_wavelet(h):
    g = np.asarray(h, dtype=np.float32)[::-1].copy()
    g[1::2] *= -1
    return g


def _fwd_block_weights(f):
    Wm = np.zeros((128, 128), np.float32)
    Wn = np.zeros((128, 128), np.float32)
    Wp = np.zeros((128, 128), np.float32)          # rows 121..127 active
    for t in range(128):
        for p in range(128):
            j = 2 * t - p
            if 0 <= j < TAPS:
                Wm[p, t] = f[j]
            j = 2 * t - p - 128
            if 0 <= j < TAPS:
                Wn[p, t] = f[j]
            j = 2 * t - p + 128
            if 0 <= j < TAPS:
                Wp[p, t] = f[j]
    return Wm, Wn, Wp


def _fwd_circ_weight(f, N):
    W = np.zeros((N, N // 2), np.float32)
    for t in range(N // 2):
        for j in range(TAPS):
            W[(2 * t - j) % N, t] += f[j]
    return W


def _inv_block_weights(f):
    We = np.zeros((128, 128), np.float32)
    Wo = np.zeros((128, 128), np.float32)
    Ww = np.zeros((128, 128), np.float32)          # rows 0..3 active
    for t in range(128):
        for p in range(128):
            j = 2 * p - t
            if 0 <= j < TAPS:
                We[p, t] = f[j]
            j = 2 * p - t - 128
            if 0 <= j < TAPS:
                Wo[p, t] = f[j]
            j = 2 * p - t + 128
            if 0 <= j < TAPS:
                Ww[p, t] = f[j]
    return We, Wo, Ww


def _inv_circ_weight(f, N):
    K = N // 2
    W = np.zeros((K, N), np.float32)
    for m in range(K):
        for n in range(N):
            if (2 * m - n) % N < TAPS:
                W[m, n] += f[(2 * m - n) % N]
    return W


def build_weights(scaling, scaling_rec):
    """Packed [128, NSLOTS*128] array (identity in last slot) + slot index."""
    filts = []
    for lvl in range(LEVELS):
        filts.append((_make_wavelet(scaling_rec[lvl]),
                      np.asarray(scaling[lvl], np.float32)))
    slots, index, seen = [], {}, {}

    def add(key, W):
        W = np.ascontiguousarray(W.astype(np.float32))
        kb = (W.shape, W.tobytes())
        if kb in seen:
            index[key] = seen[kb]
            return
        seen[kb] = len(slots)
        index[key] = len(slots)
        slots.append(W)

    add(("id",), np.eye(128, dtype=np.float32))
    for lvl in range(BIG_LVLS):
        for fi in range(2):
            Wm, Wn, Wp = _fwd_block_weights(filts[lvl][fi])
            add(("fm", lvl, fi), Wm)
            add(("fn", lvl, fi), Wn)
            add(("fp", lvl, fi), Wp)
    W5d = _fwd_circ_weight(filts[5][0], 128)
    W5a = _fwd_circ_weight(filts[5][1], 128)
    W6d = _fwd_circ_weight(filts[6][0], 64)
    W6a = _fwd_circ_weight(filts[6][1], 64)
    W7d = _fwd_circ_weight(filts[7][0], 32)
    W7a = _fwd_circ_weight(filts[7][1], 32)
    IC5d = _inv_circ_weight(filts[5][0], 128)
    IC5a = _inv_circ_weight(filts[5][1], 128)
    IC6d = _inv_circ_weight(filts[6][0], 64)
    IC6a = _inv_circ_weight(filts[6][1], 64)
    IC7d = _inv_circ_weight(filts[7][0], 32)
    IC7a = _inv_circ_weight(filts[7][1], 32)
    Wcomp = np.hstack([W5d, W5a @ W6d, W5a @ W6a @ W7d, W5a @ W6a @ W7a])
    M5 = (W5d @ IC5d + W5a @ W6d @ IC6d @ IC5a
          + W5a @ W6a @ W7d @ IC7d @ IC6a @ IC5a
          + W5a @ W6a @ W7a @ IC7a @ IC6a @ IC5a)
    D4 = _fwd_circ_weight(filts[4][0], 256)
    A5w = _fwd_circ_weight(filts[4][1], 256)
    IV4d = _inv_circ_weight(filts[4][0], 256)
    IV4a = _inv_circ_weight(filts[4][1], 256)
    C4 = A5w @ Wcomp
    M4 = D4 @ IV4d + A5w @ M5 @ IV4a
    for b in range(2):
        add(("d4", b), D4[128 * b:128 * b + 128])
        add(("c4", b), C4[128 * b:128 * b + 128])
        for h in range(2):
            add(("m4", b, h), M4[128 * b:128 * b + 128, 128 * h:128 * h + 128])
    nfwd = len(slots)
    for lvl in range(BIG_LVLS):
        for fi in range(2):
            We, Wo, Ww = _inv_block_weights(filts[lvl][fi])
            add(("ie", lvl, fi), We)
            add(("io", lvl, fi), Wo)
            add(("iw", lvl, fi), Ww)
    nslots = len(slots)
    packed = np.zeros((128, nslots * 128), np.float32)
    for i, W in enumerate(slots):
        K, M = W.shape
        packed[:K, i * 128:i * 128 + M] = W
    return packed, index, nslots, nfwd


# ---------------------------------------------------------------------------
# bass kernel
# ---------------------------------------------------------------------------

_CACHED = {}


def _build_nc(nslots, nfwd, windex):
    from contextlib import ExitStack
    import concourse.tile as tile
    from concourse import bacc, mybir

    F32 = mybir.dt.float32
    F32R = mybir.dt.float32r

    nc = bacc.Bacc("TRN2", target_bir_lowering=False, debug=False)
    x_d = nc.dram_tensor("x", (R, M_IN), F32R, kind="ExternalInput").ap()
    w_d = nc.dram_tensor("wts", (128, nslots * 128), F32R, kind="ExternalInput").ap()
    rec_d = nc.dram_tensor("recon", (R, M_IN), F32, kind="ExternalOutput").ap()
    co_d = nc.dram_tensor("coeffs", (R, L), F32, kind="ExternalOutput").ap()

    with tile.TileContext(nc) as tc, ExitStack() as ctx:
        wt_pool = ctx.enter_context(tc.tile_pool(name="wt", bufs=1))
        rm_pool = ctx.enter_context(tc.tile_pool(name="rm", bufs=2))
        xt_pool = ctx.enter_context(tc.tile_pool(name="xt", bufs=1))
        det_pool = ctx.enter_context(tc.tile_pool(name="det", bufs=1))
        app_pool = ctx.enter_context(tc.tile_pool(name="app", bufs=1))
        ia_pool = ctx.enter_context(tc.tile_pool(name="ia", bufs=1))
        win_pool = ctx.enter_context(tc.tile_pool(name="win", bufs=10))
        stg_pool = ctx.enter_context(tc.tile_pool(name="stg", bufs=6))
        cps_pool = ctx.enter_context(tc.tile_pool(name="cps", bufs=4, space="PSUM"))
        tps_pool = ctx.enter_context(tc.tile_pool(name="tps", bufs=4, space="PSUM"))

        wtile = wt_pool.tile([128, nslots * 128], F32R, tag="wt")
        # identity (slot 0) first: the in-transposes need it immediately
        nc.sync.dma_start(wtile[:, 0:128], w_d[:, 0:128])

        def wslot(key, rows=128, cols=128):
            s = windex[key]
            return wtile[0:rows, s * 128: s * 128 + cols]

        identR = wslot(("id",))

        dr_tgl = [0]

        def drain(dst, src):
            # alternate ACT / DVE to balance PSUM-drain load
            if dr_tgl[0] % 2 == 0:
                nc.scalar.copy(dst, src)
            else:
                nc.vector.tensor_copy(dst, src)
            dr_tgl[0] += 1

        def tposed(tp_slice, src, first, last, kk=128):
            nc.tensor.matmul(tp_slice.bitcast(F32R), src, identR[0:kk, 0:kk],
                             is_transpose=True, start=first, stop=last)

        # ------------------------------------------------------ phase A: load
        # xT: level-0 signal, even blocks at slots 0..15, odd at 16..31
        xT = xt_pool.tile([128, 32 * R], F32R, tag="xt")
        rms = []
        for g in range(2):
            rm = rm_pool.tile([128, L], F32R, tag="rm")
            half = M_IN // 2
            nc.sync.dma_start(rm[:, P0:P0 + half],
                              x_d[g * 128:(g + 1) * 128, 0:half])
            nc.scalar.dma_start(rm[:, P0 + half:P0 + M_IN],
                                x_d[g * 128:(g + 1) * 128, half:M_IN])
            nc.vector.tensor_copy(rm[:, 512:P0],
                                  rm[:, P0:P0 + 1].broadcast_to([128, P0 - 512]))
            nc.vector.tensor_copy(rm[:, P0 + M_IN:3584],
                                  rm[:, P0 + M_IN - 1:P0 + M_IN]
                                  .broadcast_to([128, 3584 - P0 - M_IN]))
            nc.vector.tensor_copy(rm[:, 0:512],
                                  rm[:, P0:P0 + 1].broadcast_to([128, 512]))
            nc.vector.tensor_copy(rm[:, 3584:L],
                                  rm[:, P0 + M_IN - 1:P0 + M_IN]
                                  .broadcast_to([128, L - 3584]))
            rms.append(rm)
        nc.scalar.dma_start(wtile[:, 128:nfwd * 128], w_d[:, 128:nfwd * 128])
        nc.scalar.dma_start(wtile[:, nfwd * 128:], w_d[:, nfwd * 128:])
        for g in range(2):
            for c in (2, 3, 1, 4, 5, 6, 0, 7):
                rm = rms[g]
                tp = tps_pool.tile([128, 512], F32, tag="tps")
                order = (4 * c, 4 * c + 2, 4 * c + 1, 4 * c + 3)  # E,E,O,O
                for i, b in enumerate(order):
                    tposed(tp[:, i * 128:(i + 1) * 128],
                           rm[:, b * 128:(b + 1) * 128], i == 0, i == 3)
                for h in range(2):
                    out = xT[:, h * 16 * R: (h + 1) * 16 * R] \
                        .rearrange("p (s x) -> p s x", s=16)[:, 2 * c:2 * c + 2,
                                                            g * 128:(g + 1) * 128]
                    inp = tp[:, h * 256:(h + 1) * 256] \
                        .rearrange("p (s x) -> p s x", s=2)
                    drain(out, inp)

        # coeff chunk emitter: chunk c covers coeff position blocks 4c..4c+3
        cblocks = []
        for lvl in range(BIG_LVLS):
            for b in range((2048 >> lvl) // 128):
                cblocks.append(("det", lvl, b))
        cblocks.append(("comp", None, None))
        assert len(cblocks) == 32

        def emit_coeff_chunk(c, g, dets, app8):
            tp = tps_pool.tile([128, 512], F32, tag="tps")
            nt = 0
            for i in range(4):
                kind, lvl, b = cblocks[4 * c + i]
                if kind == "det":
                    tposed(tp[:, i * 128:(i + 1) * 128],
                           dets[lvl][0][:, b * R + g * 128:
                                        b * R + (g + 1) * 128],
                           nt == 0, i == 3)
                    nt += 1
                else:
                    tposed(tp[:, i * 128:(i + 1) * 128],
                           app8[:, g * 128:(g + 1) * 128],
                           nt == 0, i == 3)
                    nt += 1
            stg = stg_pool.tile([128, 512], F32, tag="stg")
            drain(stg[:], tp[:])
            nc.sync.dma_start(co_d[g * 128:(g + 1) * 128,
                                   512 * c:512 * (c + 1)], stg[:, :])

        # --------------------------------------------------- phase B: forward
        dets = []
        cur = xT            # eo layout, fp32r
        for lvl in range(4):
            N = L >> lvl
            no = N // 2
            det = det_pool.tile([min(128, no), max(no // 128, 1) * R], F32R,
                                tag=f"det{lvl}")
            app = app_pool.tile([min(128, no), max(no // 128, 1) * R], F32R,
                                tag=f"app{lvl+1}")
            if True:
                nbO = N // 256          # blocks per E/O region
                xE = cur[:, 0:nbO * R]
                xO = cur[:, nbO * R:2 * nbO * R]
                P = max(N // 512, 1)
                w = 512 if N >= 512 else 256
                if lvl == 0:
                    q_order = [2, 3, 4, 5, 6, 1, 7, 0]
                else:
                    q_order = list(range(1, P)) + [0]
                for fi, dst in ((0, det), (1, app)):
                    for q in q_order:
                        ps = cps_pool.tile([128, 512], F32, tag="cps")
                        ps_u = ps[:, 0:w]
                        nc.tensor.matmul(ps_u, wslot(("fm", lvl, fi)),
                                         xE[:, 2 * q * R: 2 * q * R + w],
                                         start=True, stop=False)
                        nc.tensor.matmul(ps_u, wslot(("fn", lvl, fi)),
                                         xO[:, 2 * q * R: 2 * q * R + w],
                                         start=False, stop=False)
                        if q == 0:
                            nc.tensor.matmul(ps[:, 0:256], wslot(("fp", lvl, fi)),
                                             xO[:, (nbO - 1) * R: nbO * R],
                                             start=False, stop=(w == 256))
                            if w == 512:
                                nc.tensor.matmul(ps[:, 256:512],
                                                 wslot(("fp", lvl, fi)),
                                                 xO[:, 0:R],
                                                 start=False, stop=True)
                        else:
                            nc.tensor.matmul(ps_u, wslot(("fp", lvl, fi)),
                                             xO[:, (2 * q - 1) * R:
                                                (2 * q + 1) * R],
                                             start=False, stop=True)
                        if fi == 0:
                            drain(det[:, 2 * q * R: 2 * q * R + w], ps_u)
                        elif (lvl + 1) < BIG_LVLS:
                            # app is eo: halves go to E/O regions, slot q
                            nbO2 = no // 256
                            out = app[:].rearrange("p (h s x) -> p h s x",
                                                   h=2, s=nbO2)
                            drain(out[:, :, q, :],
                                  ps_u.rearrange("p (h x) -> p h x", h=2))
                        else:
                            drain(app[:, 0:w], ps_u)
            dets.append((det, no))
            cur = app

        app4 = cur          # eo layout [128, 2*R]: E = [0:R], O = [R:2R]
        # composed levels 4..7: det4, [det5|det6|det7|app8], and the
        # reconstructed level-4 approx, each straight from app4
        ia4 = ia_pool.tile([128, 2 * R], F32R, tag="ia4")
        ps = cps_pool.tile([128, 512], F32, tag="cps")
        for h in range(2):
            for b in range(2):
                nc.tensor.matmul(ps[:, h * R:(h + 1) * R], wslot(("m4", b, h)),
                                 app4[:, b * R:(b + 1) * R],
                                 start=(h == 0 and b == 0),
                                 stop=(h == 1 and b == 1))
        drain(ia4[:], ps[:])
        det4 = det_pool.tile([128, R], F32R, tag="det4")
        ps = cps_pool.tile([128, 512], F32, tag="cps")
        nc.tensor.matmul(ps[:, 0:R], wslot(("d4", 0)), app4[:, 0:R],
                         start=True, stop=False)
        nc.tensor.matmul(ps[:, 0:R], wslot(("d4", 1)), app4[:, R:2 * R],
                         start=False, stop=True)
        drain(det4[:], ps[:, 0:R])
        dets.append((det4, 128))
        comp = det_pool.tile([128, R], F32R, tag="comp")
        ps = cps_pool.tile([128, 512], F32, tag="cps")
        nc.tensor.matmul(ps[:, 0:R], wslot(("c4", 0)), app4[:, 0:R],
                         start=True, stop=False)
        nc.tensor.matmul(ps[:, 0:R], wslot(("c4", 1)), app4[:, R:2 * R],
                         start=False, stop=True)
        drain(comp[:], ps[:, 0:R])

        for g in range(2):
            for cch in range(8):
                emit_coeff_chunk(cch, g, dets, comp)

        # --------------------------------------------------- phase C: inverse
        a = ia4
        for lvl in range(3, -1, -1):
            N = L >> lvl
            K = N // 2
            d = dets[lvl][0]
            nb = N // 128
            nbK = max(K // 128, 1)
            if lvl > 0:
                out = ia_pool.tile([128, nb * R], F32R, tag=f"ia{lvl}")
                outv = out[:].rearrange("p (s x) -> p s x", s=nb)
            if N == 256:
                ps = cps_pool.tile([128, 512], F32, tag="cps")
                nc.tensor.matmul(ps[:, 0:256], wslot(("ie", lvl, 0)),
                                 d[:, 0:R], start=True, stop=False)
                nc.tensor.matmul(ps[:, 0:256], wslot(("ie", lvl, 1)),
                                 a[:, 0:R], start=False, stop=False)
                nc.tensor.matmul(ps[:, 256:512], wslot(("io", lvl, 0)),
                                 d[:, 0:R], start=False, stop=False)
                nc.tensor.matmul(ps[:, 256:512], wslot(("io", lvl, 1)),
                                 a[:, 0:R], start=False, stop=False)
                nc.tensor.matmul(ps[:, 256:512], wslot(("iw", lvl, 0)),
                                 d[:, 0:R], start=False, stop=False)
                nc.tensor.matmul(ps[:, 256:512], wslot(("iw", lvl, 1)),
                                 a[:, 0:R], start=False, stop=True)
                drain(out[:, 0:512], ps[:])
                a = out
                continue
            qr = range(1, 7) if lvl == 0 else range(nb // 4)
            for q in qr:
                # even pair: out blocks (4q, 4q+2) <- d/a blocks 2q, 2q+1
                pse = cps_pool.tile([128, 512], F32, tag="cps")
                nc.tensor.matmul(pse[:], wslot(("ie", lvl, 0)),
                                 d[:, 2 * q * R:(2 * q + 2) * R],
                                 start=True, stop=False)
                nc.tensor.matmul(pse[:], wslot(("ie", lvl, 1)),
                                 a[:, 2 * q * R:(2 * q + 2) * R],
                                 start=False, stop=True)
                # odd pair: out blocks (4q+1, 4q+3)
                pso = cps_pool.tile([128, 512], F32, tag="cps")
                nc.tensor.matmul(pso[:], wslot(("io", lvl, 0)),
                                 d[:, 2 * q * R:(2 * q + 2) * R],
                                 start=True, stop=False)
                nc.tensor.matmul(pso[:], wslot(("io", lvl, 1)),
                                 a[:, 2 * q * R:(2 * q + 2) * R],
                                 start=False, stop=False)
                if 2 * q + 2 < nbK:
                    nc.tensor.matmul(pso[:], wslot(("iw", lvl, 0)),
                                     d[:, (2 * q + 1) * R:(2 * q + 3) * R],
                                     start=False, stop=False)
                    nc.tensor.matmul(pso[:], wslot(("iw", lvl, 1)),
                                     a[:, (2 * q + 1) * R:(2 * q + 3) * R],
                                     start=False, stop=True)
                else:
                    nc.tensor.matmul(pso[:, 0:256], wslot(("iw", lvl, 0)),
                                     d[:, (2 * q + 1) * R:(2 * q + 2) * R],
                                     start=False, stop=False)
                    nc.tensor.matmul(pso[:, 0:256], wslot(("iw", lvl, 1)),
                                     a[:, (2 * q + 1) * R:(2 * q + 2) * R],
                                     start=False, stop=False)
                    nc.tensor.matmul(pso[:, 256:512], wslot(("iw", lvl, 0)),
                                     d[:, 0:R], start=False, stop=False)
                    nc.tensor.matmul(pso[:, 256:512], wslot(("iw", lvl, 1)),
                                     a[:, 0:R], start=False, stop=True)
                if lvl > 0:
                    drain(outv[:, 4 * q:4 * q + 3:2, :],
                          pse[:].rearrange("p (s x) -> p s x", s=2))
                    drain(outv[:, 4 * q + 1:4 * q + 4:2, :],
                          pso[:].rearrange("p (s x) -> p s x", s=2))
                else:
                    we = win_pool.tile([128, 512], F32R, tag="win")
                    wo = win_pool.tile([128, 512], F32R, tag="win")
                    drain(we[:], pse[:])
                    drain(wo[:], pso[:])
                    for g in range(2):
                        tp = tps_pool.tile([128, 512], F32, tag="tps")
                        srcs = [we[:, g * 128:g * 128 + 128],
                                wo[:, g * 128:g * 128 + 128],
                                we[:, 256 + g * 128:256 + g * 128 + 128],
                                wo[:, 256 + g * 128:256 + g * 128 + 128]]
                        for i, s in enumerate(srcs):
                            tposed(tp[:, i * 128:(i + 1) * 128], s,
                                   i == 0, i == 3)
                        stg = stg_pool.tile([128, 512], F32, tag="stg")
                        drain(stg[:], tp[:])
                        lo = 512 * q
                        s0 = max(P0 - lo, 0)
                        s1 = min(P0 + M_IN - lo, 512)
                        nc.sync.dma_start(
                            rec_d[g * 128:(g + 1) * 128,
                                  lo + s0 - P0: lo + s1 - P0],
                            stg[:, s0:s1])
            if lvl > 0:
                a = out

    nc.compile()
    return nc


def kernel(input, scaling, scaling_rec):
    from concourse import bass_utils

    x = np.ascontiguousarray(np.asarray(input, dtype=np.float32))
    packed, windex, nslots, nfwd = build_weights(
        np.asarray(scaling, np.float32), np.asarray(scaling_rec, np.float32))
    key = (nslots, nfwd, tuple(sorted((k, v) for k, v in windex.items())))
    if key not in _CACHED:
        _CACHED[key] = _build_nc(nslots, nfwd, windex)
    nc = _CACHED[key]

    in_maps = []
    for c in range(NCORES):
        in_maps.append({"x": np.ascontiguousarray(x[c * R:(c + 1) * R]),
                        "wts": packed})
    res = bass_utils.run_bass_kernel_spmd(nc, in_maps, core_ids=list(range(NCORES)))
    recon = np.concatenate([r["recon"] for r in res.results], axis=0)
    coeffs = np.concatenate([r["coeffs"] for r in res.results], axis=0)
    return recon, coeffs


if __name__ == "__main__":
    rng = np.random.default_rng(0)
    x = rng.standard_normal((NCORES * R, M_IN), dtype=np.float32)
    DEC_HI = np.array([-0.23037781330885523, 0.7148465705525415, -0.6308807679295904,
                       -0.02798376941698385, 0.18703481171888114, 0.030841381835986965,
                       -0.032883011666982945, -0.010597401784997278], dtype=np.float32)
    sc = np.tile(DEC_HI, (LEVELS, 1))
    scr = np.tile(DEC_HI[::-1], (LEVELS, 1))
    r, c = kernel(x, sc, scr)
    print("recon", r.shape, "coeffs", c.shape)
